# revision 13
# baseline (speedup 1.0000x reference)
"""DGMNet (dense MLP, 4 DGM layers) Trainium2 kernel.

Strategy: data-parallel over the batch dim (65536 rows -> 8 cores x 8192).
Inside each core, activations live feature-major in SBUF ([128 features x
batch-tile] tiles), so every gate matmul is out[M=feat,N=batch] =
W.T-slice @ S with PE accumulation over the 1024-feature contraction, the
x-side projections (K=16) are folded into the same PSUM accumulation
group (issued 4-wide via row tiling so they cost ~1/4), and biases ride
the ACT tanh for free. The scalar output row of tile t is computed during
tile t+1's S1/G phase so the PE never waits on the layer-3 tail.

Host-side preprocessing (numpy): transpose x and the weight matrices so
the kernel never transposes on-device, replicate the 16-row U block at
partitions 0/32/64/96 for row-tiled matmuls, and fold the U/W bias pairs.
"""

import sys

sys.path.insert(0, "/opt/trn_rl_repo")

import numpy as np

B_FULL = 65536
KI = 16
H = 1024
NCORES = 8
BC = B_FULL // NCORES  # per-core batch (8192)
NB = 512               # batch tile (one PSUM bank of fp32)
NM = H // 128          # feature tiles (8)
N_LAYERS = 4

# float32r streams fp32 data through the PE at 1 column/cycle (4x the fp32
# rate) with reduced internal mantissa; float32 is the exact-but-slow mode.
MM_DT = "float32r"

_BUILD_CACHE = {}


def _build(bc, nb, mm_dt, repeat=1, pack_k16=True, out_slot="s1", s1_act=True, share_wgs=True):
    """Build + compile the single-core Bass program. Returns nc.

    repeat > 1 re-runs the whole computation (for slope-based timing of the
    device execution under the large axon dispatch overhead)."""
    import concourse.bacc as bacc
    import concourse.mybir as mybir
    import concourse.tile as tile

    f32 = mybir.dt.float32
    mdt = getattr(mybir.dt, mm_dt)
    Tanh = mybir.ActivationFunctionType.Tanh
    Ident = mybir.ActivationFunctionType.Identity
    mult = mybir.AluOpType.mult
    add = mybir.AluOpType.add

    nt = bc // nb

    nc = bacc.Bacc("TRN2", target_bir_lowering=False, debug=False,
                   num_devices=NCORES)

    xT_d = nc.dram_tensor("xT", [KI, bc], mdt, kind="ExternalInput").ap()
    wz_d = nc.dram_tensor("WzT", [H, H], mdt, kind="ExternalInput").ap()
    wg_d = nc.dram_tensor("WgT", [H, H], mdt, kind="ExternalInput").ap()
    u_d = nc.dram_tensor("U", [128, 5 * H], mdt, kind="ExternalInput").ap()
    bias_d = nc.dram_tensor("BIAS", [128, 48], f32, kind="ExternalInput").ap()
    ow_d = nc.dram_tensor("OW", [128, NM], mdt, kind="ExternalInput").ap()
    y_d = nc.dram_tensor("Y", [1, bc], f32, kind="ExternalOutput").ap()

    with tile.TileContext(nc) as tc:
        with (
            tc.tile_pool(name="const", bufs=1) as cpool,
            tc.tile_pool(name="xt", bufs=2) as xt_pool,
            tc.tile_pool(name="s", bufs=2) as s_pool,
            tc.tile_pool(name="act", bufs=1) as act_pool,
            tc.tile_pool(name="ov", bufs=2) as ov_pool,
            tc.tile_pool(name="psum", bufs=7, space="PSUM") as ps_pool,
            tc.tile_pool(name="pso", bufs=1, space="PSUM") as pso_pool,
        ):
            # ---- resident constants (small ones first; G needs Wg before
            # Z needs Wz, so load Wg ahead of Wz) -------------------------
            u_sb = cpool.tile([128, 5 * H], mdt)
            nc.gpsimd.dma_start(u_sb[:], u_d[:])
            bias_sb = cpool.tile([128, 48], f32)
            nc.gpsimd.dma_start(bias_sb[:], bias_d[:])
            ow_sb = cpool.tile([128, NM], mdt)
            nc.gpsimd.dma_start(ow_sb[:], ow_d[:])
            wz_sb = cpool.tile([128, NM * H], mdt)
            wg_sb = cpool.tile([128, NM * H], mdt)
            for k in range(NM):
                nc.gpsimd.dma_start(wg_sb[:, k * H:(k + 1) * H],
                                    wg_d[k * 128:(k + 1) * 128, :])
            for k in range(NM):
                nc.gpsimd.dma_start(wz_sb[:, k * H:(k + 1) * H],
                                    wz_d[k * 128:(k + 1) * 128, :])

            def w_ap(w_sb, k, m):
                return w_sb[:, k * H + m * 128:k * H + (m + 1) * 128]

            def u_ap(g, m, c):
                return u_sb[32 * c:32 * c + KI,
                            g * H + m * 128:g * H + (m + 1) * 128]

            def b_ap(g, m):
                return bias_sb[:, g * NM + m:g * NM + m + 1]

            def x_starts(gate, xt, pss, single):
                """Row-tiled (4-concurrent) K=16 start matmuls for m-quads."""
                for mq in (0, 4):
                    for c in range(4):
                        m = mq + c
                        cc = c if pack_k16 else 0
                        nc.tensor.matmul(
                            pss[m][:], u_ap(gate, m, cc),
                            xt[32 * cc:32 * cc + KI, :],
                            start=True, stop=single,
                            tile_position=(32 * cc, 0))

            # ---- per batch tile -----------------------------------------
            pend = None  # deferred output row of the previous batch tile

            def emit_out(pend):
                h_prev, tp, up = pend
                po = pso_pool.tile([1, nb], f32, tag="po", name=f"po_{up}")
                for k in range(NM):
                    nc.tensor.matmul(po[:], ow_sb[:, k:k + 1], h_prev[k][:],
                                     start=(k == 0), stop=(k == NM - 1))
                orow = ov_pool.tile([1, nb], f32, tag="orow", name=f"orow_{up}")
                nc.vector.tensor_scalar_add(orow[:], po[:],
                                            bias_sb[0:1, 40:41])
                nc.gpsimd.dma_start(y_d[0:1, tp * nb:(tp + 1) * nb], orow[:])

            for rep in range(repeat):
                for t in range(nt):
                    t_u = rep * nt + t  # unique suffix for tile names
                    xt = xt_pool.tile([128, nb], mdt, tag="xt",
                                      name=f"xt_{t_u}")
                    for c in range(4):
                        nc.gpsimd.dma_start(xt[32 * c:32 * c + KI, :],
                                            xT_d[:, t * nb:(t + 1) * nb])

                    # S1 = x @ Sw.T + Sw_b (no tanh; evacuate via ACT
                    # Identity so the DVE stays free for the combines)
                    s_cur = [s_pool.tile([128, nb], mdt, tag=f"s{k}",
                                         name=f"s_{t_u}_0_{k}")
                             for k in range(NM)]
                    ps1 = [ps_pool.tile([128, nb], f32, tag="ps",
                                        name=f"ps_s1_{t_u}_{m}")
                           for m in range(NM)]
                    x_starts(0, xt, ps1, single=True)
                    for m in range(NM):
                        if s1_act:
                            nc.scalar.activation(s_cur[m][:], ps1[m][:],
                                                 Ident, bias=b_ap(0, m))
                        else:
                            nc.vector.tensor_scalar_add(s_cur[m][:],
                                                        ps1[m][:], b_ap(0, m))

                    # previous tile's output row fills the S1->G dep gap
                    if pend is not None and out_slot == "s1":
                        emit_out(pend)
                        pend = None

                    # wgS1 = Wg @ S1, shared by G and the layer-0 R
                    # gate (the reference reuses it); parked in the h slots,
                    # which are free until layer 0's H.
                    if share_wgs:
                        wgs = [act_pool.tile([128, nb], mdt, tag=f"h{m}",
                                             name=f"wgs_{t_u}_{m}")
                               for m in range(NM)]
                        for m in range(NM):
                            ps = ps_pool.tile([128, nb], f32, tag="ps",
                                              name=f"ps_wgs_{t_u}_{m}")
                            for k in range(NM):
                                nc.tensor.matmul(ps[:], w_ap(wg_sb, k, m),
                                                 s_cur[k][:],
                                                 start=(k == 0),
                                                 stop=(k == NM - 1))
                            nc.scalar.activation(wgs[m][:], ps[:], Ident)
    

                    if pend is not None and out_slot == "g":
                        emit_out(pend)
                        pend = None

                    for i in range(N_LAYERS):
                        # R = tanh(br + Ur x + Wg S); layer 0 reuses wgS1
                        r_t = [act_pool.tile([128, nb], mdt, tag=f"r{m}",
                                             name=f"r_{t_u}_{i}_{m}")
                               for m in range(NM)]
                        if i == 0 and not share_wgs:
                            # unshared fallback: full Wg@S1 groups for G and R
                            g_t = [act_pool.tile([128, nb], f32, tag=f"g{m}",
                                                 name=f"g_{t_u}_{m}")
                                   for m in range(NM)]
                            for gate, dest, bgi in ((2, g_t, 2), (3, r_t, 3)):
                                for mq in (0, 4):
                                    pss = {mq + c: ps_pool.tile(
                                        [128, nb], f32, tag="ps",
                                        name=f"ps_u{gate}_{t_u}_{mq + c}")
                                        for c in range(4)}
                                    for c in range(4):
                                        m = mq + c
                                        cc = c if pack_k16 else 0
                                        nc.tensor.matmul(
                                            pss[m][:], u_ap(gate, m, cc),
                                            xt[32 * cc:32 * cc + KI, :],
                                            start=True, stop=False,
                                            tile_position=(32 * cc, 0))
                                    for c in range(4):
                                        m = mq + c
                                        for k in range(NM):
                                            nc.tensor.matmul(
                                                pss[m][:], w_ap(wg_sb, k, m),
                                                s_cur[k][:],
                                                start=False,
                                                stop=(k == NM - 1))
                                        nc.scalar.activation(
                                            dest[m][:], pss[m][:], Tanh,
                                            bias=b_ap(bgi, m))
                        elif i == 0:
                            ps_r = [ps_pool.tile([128, nb], f32, tag="ps",
                                                 name=f"ps_r_{t_u}_0_{m}")
                                    for m in range(NM)]
                            x_starts(3, xt, ps_r, single=True)
                            for m in range(NM):
                                nc.vector.tensor_add(r_t[m][:], ps_r[m][:],
                                                     wgs[m][:])
                                nc.scalar.activation(r_t[m][:], r_t[m][:],
                                                     Tanh, bias=b_ap(3, m))
                            # G = tanh(bg + Ug x + wgS1); the (1-G) transform
                            # is deferred past the H matmuls to keep the DVE
                            # off the H-gate critical path
                            g_t = [act_pool.tile([128, nb], f32, tag=f"g{m}",
                                                 name=f"g_{t_u}_{m}")
                                   for m in range(NM)]
                            ps_g = [ps_pool.tile([128, nb], f32, tag="ps",
                                                 name=f"ps_g_{t_u}_{m}")
                                    for m in range(NM)]
                            x_starts(2, xt, ps_g, single=True)
                            for m in range(NM):
                                nc.vector.tensor_add(g_t[m][:], ps_g[m][:],
                                                     wgs[m][:])
                                nc.scalar.activation(g_t[m][:], g_t[m][:],
                                                     Tanh, bias=b_ap(2, m))
                        else:
                            for mq in (0, 4):
                                pss = {mq + c: ps_pool.tile(
                                    [128, nb], f32, tag="ps",
                                    name=f"ps_r_{t_u}_{i}_{mq + c}")
                                    for c in range(4)}
                                for c in range(4):
                                    m = mq + c
                                    cc = c if pack_k16 else 0
                                    nc.tensor.matmul(
                                        pss[m][:], u_ap(3, m, cc),
                                        xt[32 * cc:32 * cc + KI, :],
                                        start=True, stop=False,
                                        tile_position=(32 * cc, 0))
                                for c in range(4):
                                    m = mq + c
                                    for k in range(NM):
                                        nc.tensor.matmul(
                                            pss[m][:], w_ap(wg_sb, k, m),
                                            s_cur[k][:],
                                            start=False, stop=(k == NM - 1))
                                    nc.scalar.activation(r_t[m][:], pss[m][:],
                                                         Tanh, bias=b_ap(3, m))

                        # Z = tanh(bz + Uz x + Wz S)
                        z_t = [act_pool.tile([128, nb], f32, tag=f"z{m}",
                                             name=f"z_{t_u}_{i}_{m}")
                               for m in range(NM)]
                        for mq in (0, 4):
                            pss = {mq + c: ps_pool.tile(
                                [128, nb], f32, tag="ps",
                                name=f"ps_z_{t_u}_{i}_{mq + c}")
                                for c in range(4)}
                            for c in range(4):
                                m = mq + c
                                cc = c if pack_k16 else 0
                                nc.tensor.matmul(pss[m][:], u_ap(1, m, cc),
                                                 xt[32 * cc:32 * cc + KI, :],
                                                 start=True, stop=False,
                                                 tile_position=(32 * cc, 0))
                            for c in range(4):
                                m = mq + c
                                for k in range(NM):
                                    nc.tensor.matmul(
                                        pss[m][:], w_ap(wz_sb, k, m),
                                        s_cur[k][:],
                                        start=False, stop=(k == NM - 1))
                                nc.scalar.activation(z_t[m][:], pss[m][:],
                                                     Tanh, bias=b_ap(1, m))

                        # SR = S * R, in place into R's tiles
                        for k in range(NM):
                            nc.vector.tensor_mul(r_t[k][:], s_cur[k][:],
                                                 r_t[k][:])

                        # H = tanh(bh + Uh x + Wg (S*R))
                        h_t = [act_pool.tile([128, nb], mdt, tag=f"h{m}",
                                             name=f"h_{t_u}_{i}_{m}")
                               for m in range(NM)]
                        for mq in (0, 4):
                            pss = {mq + c: ps_pool.tile(
                                [128, nb], f32, tag="ps",
                                name=f"ps_h_{t_u}_{i}_{mq + c}")
                                for c in range(4)}
                            for c in range(4):
                                m = mq + c
                                cc = c if pack_k16 else 0
                                nc.tensor.matmul(pss[m][:], u_ap(4, m, cc),
                                                 xt[32 * cc:32 * cc + KI, :],
                                                 start=True, stop=False,
                                                 tile_position=(32 * cc, 0))
                            for c in range(4):
                                m = mq + c
                                for k in range(NM):
                                    nc.tensor.matmul(
                                        pss[m][:], w_ap(wg_sb, k, m),
                                        r_t[k][:],
                                        start=False, stop=(k == NM - 1))
                                nc.scalar.activation(h_t[m][:], pss[m][:],
                                                     Tanh, bias=b_ap(4, m))

                        if i == 0:
                            # deferred (1 - G), now that H's matmuls are in
                            # flight
                            for m in range(NM):
                                nc.vector.tensor_scalar(g_t[m][:], g_t[m][:],
                                                        -1.0, 1.0,
                                                        op0=mult, op1=add)

                        # output = (1-G)*H + Z*S  (h <- (1-G)*h; z <- z*s;
                        # h += z)
                        for m in range(NM):
                            nc.vector.tensor_mul(h_t[m][:], g_t[m][:],
                                                 h_t[m][:])
                            nc.vector.tensor_mul(z_t[m][:], z_t[m][:],
                                                 s_cur[m][:])
                            nc.vector.tensor_add(h_t[m][:], h_t[m][:],
                                                 z_t[m][:])

                        if i < N_LAYERS - 1:
                            s_new = [s_pool.tile([128, nb], mdt, tag=f"s{k}",
                                                 name=f"s_{t_u}_{i + 1}_{k}")
                                     for k in range(NM)]
                            for m in range(NM):
                                nc.scalar.activation(s_new[m][:], h_t[m][:],
                                                     Tanh)
                            s_cur = s_new

                    # y = out_w @ output + out_b, deferred into the next
                    # tile's S1/G phase
                    pend = (h_t, t, t_u)
                    if out_slot == "end":
                        emit_out(pend)
                        pend = None

            if pend is not None:
                emit_out(pend)

    nc.compile()
    return nc


def _get_nc(bc=BC, nb=NB, mm_dt=MM_DT):
    key = (bc, nb, mm_dt)
    if key not in _BUILD_CACHE:
        _BUILD_CACHE[key] = _build(bc, nb, mm_dt)
    return _BUILD_CACHE[key]


def _prep_inputs(x, Sw_w, Sw_b, Uz_w, Uz_b, Wz_w, Wz_b, Ug_w, Ug_b, Wg_w,
                 Wg_b, Ur_w, Ur_b, Uh_w, Uh_b, out_w, out_b):
    f = np.float32
    xT = np.ascontiguousarray(np.asarray(x, f).T)               # [16, B]
    WzT = np.ascontiguousarray(np.asarray(Wz_w, f).T)           # [H, H]
    WgT = np.ascontiguousarray(np.asarray(Wg_w, f).T)
    U16 = np.concatenate(
        [np.asarray(w, f).T for w in (Sw_w, Uz_w, Ug_w, Ur_w, Uh_w)],
        axis=1)                                                 # [16, 5H]
    U = np.zeros((128, 5 * H), f)
    for c in range(4):
        U[32 * c:32 * c + KI] = U16
    bias = np.zeros((128, 48), f)
    combos = [
        np.asarray(Sw_b, f),
        np.asarray(Uz_b, f) + np.asarray(Wz_b, f),
        np.asarray(Ug_b, f) + np.asarray(Wg_b, f),
        np.asarray(Ur_b, f) + np.asarray(Wg_b, f),
        np.asarray(Uh_b, f) + np.asarray(Wg_b, f),
    ]
    for g, b in enumerate(combos):
        bias[:, g * NM:(g + 1) * NM] = b.reshape(NM, 128).T
    bias[:, 40] = np.float32(np.asarray(out_b, f)[0])
    OW = np.ascontiguousarray(np.asarray(out_w, f).reshape(NM, 128).T)
    return xT, WzT, WgT, U, bias, OW


def kernel(**inputs):
    from concourse.bass_utils import run_bass_kernel_spmd

    nc = _get_nc()
    in_maps = _make_in_maps(inputs)
    res = run_bass_kernel_spmd(nc, in_maps, list(range(NCORES)))
    y = np.concatenate([res.results[c]["Y"] for c in range(NCORES)], axis=1)
    return np.ascontiguousarray(y.reshape(B_FULL, 1)).astype(np.float32)


def _make_in_maps(inputs):
    xT, WzT, WgT, U, bias, OW = _prep_inputs(**inputs)
    return [{
        "xT": np.ascontiguousarray(xT[:, c * BC:(c + 1) * BC]),
        "WzT": WzT, "WgT": WgT, "U": U, "BIAS": bias, "OW": OW,
    } for c in range(NCORES)]


def timed_run(inputs, iters=5, nc=None, pipeline=1):
    """Build a persistent jitted runner (so walrus compiles once), stage the
    inputs on-device, and time repeated executions. Returns (best_ns,
    all_ns, output)."""
    import time
    import jax
    from jax.sharding import Mesh, PartitionSpec, NamedSharding
    from jax.experimental.shard_map import shard_map
    from concourse import bass2jax, mybir

    bass2jax.install_neuronx_cc_hook()
    if nc is None:
        nc = _get_nc()
    in_maps = _make_in_maps(inputs)
    n_cores = NCORES

    partition_name = (nc.partition_id_tensor.name
                      if nc.partition_id_tensor else None)
    in_names, out_names, out_avals, zero_outs = [], [], [], []
    for alloc in nc.m.functions[0].allocations:
        if not isinstance(alloc, mybir.MemoryLocationSet):
            continue
        name = alloc.memorylocations[0].name
        if alloc.kind == "ExternalInput":
            if name != partition_name:
                in_names.append(name)
        elif alloc.kind == "ExternalOutput":
            shape = tuple(alloc.tensor_shape)
            dtype = mybir.dt.np(alloc.dtype)
            out_names.append(name)
            out_avals.append(jax.core.ShapedArray(shape, dtype))
            zero_outs.append(np.zeros(shape, dtype))
    n_params = len(in_names)
    n_outs = len(out_avals)
    all_in = list(in_names) + list(out_names)
    if partition_name is not None:
        all_in.append(partition_name)
    donate = tuple(range(n_params, n_params + n_outs))

    def _body(*args):
        operands = list(args)
        if partition_name is not None:
            operands.append(bass2jax.partition_id_tensor())
        outs = bass2jax._bass_exec_p.bind(
            *operands,
            out_avals=tuple(out_avals),
            in_names=tuple(all_in),
            out_names=tuple(out_names),
            lowering_input_output_aliases=(),
            sim_require_finite=True,
            sim_require_nnan=True,
            nc=nc,
        )
        return tuple(outs)

    devices = jax.devices()[:n_cores]
    mesh = Mesh(np.asarray(devices), ("core",))
    spec = PartitionSpec("core")
    sharded = jax.jit(
        shard_map(_body, mesh=mesh, in_specs=(spec,) * (n_params + n_outs),
                  out_specs=(spec,) * n_outs, check_rep=False),
        donate_argnums=donate, keep_unused=True)

    sharding = NamedSharding(mesh, spec)
    dev_in = [
        jax.device_put(
            np.concatenate([np.asarray(in_maps[c][n]) for c in range(n_cores)],
                           axis=0), sharding)
        for n in in_names
    ]
    def fresh_zeros():
        return [np.zeros((n_cores * z.shape[0], *z.shape[1:]), z.dtype)
                for z in zero_outs]

    # warmup (compiles)
    outs = sharded(*dev_in, *fresh_zeros())
    jax.block_until_ready(outs)

    state = {"outs": outs}

    def run_once(pipeline_n=pipeline):
        zss = [fresh_zeros() for _ in range(pipeline_n)]
        t0 = time.perf_counter()
        all_outs = [sharded(*dev_in, *zs) for zs in zss]
        jax.block_until_ready(all_outs)
        state["outs"] = all_outs[-1]
        return int((time.perf_counter() - t0) * 1e9 / pipeline_n)

    def get_y():
        y = np.asarray(state["outs"][out_names.index("Y")])  # [8, BC]
        return np.ascontiguousarray(
            y.reshape(1, B_FULL).reshape(B_FULL, 1)).astype(np.float32)

    if iters is None:
        return run_once, get_y

    times = [run_once() for _ in range(iters)]
    return min(times), times, get_y()
